# revision 1
# baseline (speedup 1.0000x reference)
# MoE grouped-GEMM kernel for Trainium2 (Bass/Tile), 8 NeuronCores SPMD.
#
# Problem: x [65536, 1024] fp32, 64 experts each owning a contiguous group of
# 1024 tokens. Per expert: h = relu(x_g @ W1^T) (1024->64), y_g = h @ W2^T
# (64->1024).
#
# Sharding: expert-parallel == token-parallel here (tokens pre-sorted by
# expert, equal groups). Core c handles experts 8c..8c+7 and their 8192
# tokens. No collectives needed; host slices inputs and concatenates outputs.
#
# Per-core kernel (memory-bound; ~68 MB HBM traffic/core):
#   for each local expert e:
#     x^T tile  [128, 8, 1024]  <- one 4 MB contiguous DMA (host pre-permuted)
#     FC1: 16 matmuls  psum_h[64, 1024] += w1T[128,64].T @ xT[128,512]
#     ReLU (ACT engine) psum -> sbuf h [64, 1024]
#     FC2: per 128-token chunk m: psum_y[128,1024] = h[:,m*128:+128].T @ w2T
#     DVE copy psum_y -> y tile [128, 8, 1024]; one 4 MB DMA store
import numpy as np

import concourse.bacc as bacc
import concourse.bass as bass
import concourse.mybir as mybir
import concourse.tile as tile
from concourse.bass import ds, ts
from concourse.bass_utils import run_bass_kernel_spmd

E = 64          # experts
H = 64          # expert hidden
D = 1024        # d_in
O = 1024        # d_out
T = 65536       # total tokens
N_CORES = 8
E_PER = E // N_CORES        # 8 experts per core
TPE = T // E                # 1024 tokens per expert
T_PER = TPE * E_PER         # 8192 tokens per core
DC = D // 128               # 8 contraction chunks of 128
MC = TPE // 128             # 8 token chunks of 128 per expert
FP32 = mybir.dt.float32

_NC_CACHE = {}


def build_nc():
    nc = bacc.Bacc("TRN2", target_bir_lowering=False, debug=False,
                   num_devices=N_CORES)

    xs = nc.declare_dram_parameter("xs", [E_PER, 128, DC, TPE], FP32,
                                   isOutput=False)
    w1s = nc.declare_dram_parameter("w1s", [E_PER, 128, DC * H], FP32,
                                    isOutput=False)
    w2s = nc.declare_dram_parameter("w2s", [E_PER, H, O], FP32,
                                    isOutput=False)
    y = nc.declare_dram_parameter("y", [T_PER, O], FP32, isOutput=True)

    with tile.TileContext(nc) as tc:
        with (
            tc.tile_pool(name="wpool", bufs=1) as wpool,
            tc.tile_pool(name="xpool", bufs=2) as xpool,
            tc.tile_pool(name="hpool", bufs=2) as hpool,
            tc.tile_pool(name="ypool", bufs=2) as ypool,
            tc.tile_pool(name="phpool", bufs=2, space=bass.MemorySpace.PSUM) as phpool,
            tc.tile_pool(name="pypool", bufs=2, space=bass.MemorySpace.PSUM) as pypool,
        ):
            # All expert weights resident in SBUF for the whole kernel.
            w1_t = wpool.tile([128, E_PER, DC * H], FP32, tag="w1")
            w2_t = wpool.tile([H, E_PER, O], FP32, tag="w2")
            for le in range(E_PER):
                nc.sync.dma_start(out=w1_t[:, le, :], in_=w1s[le])
                nc.sync.dma_start(out=w2_t[:, le, :], in_=w2s[le])

            for le in range(E_PER):
                x_t = xpool.tile([128, DC, TPE], FP32, tag="x")
                nc.sync.dma_start(out=x_t[:], in_=xs[le])

                # FC1: psum_h[h, t] = sum_d W1[h, d] * x[t, d]
                ph = phpool.tile([H, TPE], FP32, tag="ph")
                for tb in range(TPE // 512):
                    for dc in range(DC):
                        nc.tensor.matmul(
                            ph[:, ts(tb, 512)],
                            w1_t[:, le, ds(dc * H, H)],   # lhsT [128, 64]
                            x_t[:, dc, ts(tb, 512)],      # rhs  [128, 512]
                            start=(dc == 0),
                            stop=(dc == DC - 1),
                        )

                h_t = hpool.tile([H, TPE], FP32, tag="h")
                nc.scalar.activation(h_t[:], ph[:],
                                     mybir.ActivationFunctionType.Relu)

                # FC2: y[t, o] = sum_h h[h, t] * W2[o, h]
                y_t = ypool.tile([128, MC, O], FP32, tag="y")
                for m in range(MC):
                    py = pypool.tile([128, O], FP32, tag="py")
                    for oc in range(O // 512):
                        nc.tensor.matmul(
                            py[:, ts(oc, 512)],
                            h_t[:, ts(m, 128)],           # lhsT [64, 128]
                            w2_t[:, le, ts(oc, 512)],     # rhs  [64, 512]
                            start=True,
                            stop=True,
                        )
                    nc.vector.tensor_copy(y_t[:, m, :], py[:])

                y_view = y[ds(le * TPE, TPE), :].rearrange(
                    "(m p) o -> p m o", p=128)
                nc.sync.dma_start(out=y_view, in_=y_t[:])

    nc.compile()
    return nc


def get_nc():
    if "nc" not in _NC_CACHE:
        _NC_CACHE["nc"] = build_nc()
    return _NC_CACHE["nc"]


def _prep_inputs(x, batched_fc1_w, batched_fc2_w):
    x = np.ascontiguousarray(np.asarray(x, dtype=np.float32))
    fc1 = np.ascontiguousarray(np.asarray(batched_fc1_w, dtype=np.float32))
    fc2 = np.ascontiguousarray(np.asarray(batched_fc2_w, dtype=np.float32))

    # xs[e, p, c, t] = x[e*TPE + t, c*128 + p]
    xs = np.ascontiguousarray(
        x.reshape(E, TPE, DC, 128).transpose(0, 3, 2, 1))
    # w1s[e, p, c*H + h] = W1[e, h, c*128 + p]
    w1s = np.ascontiguousarray(
        fc1.reshape(E, H, DC, 128).transpose(0, 3, 2, 1).reshape(E, 128, DC * H))
    # w2s[e, h, o] = W2[e, o, h]
    w2s = np.ascontiguousarray(fc2.transpose(0, 2, 1))

    in_maps = []
    for c in range(N_CORES):
        sl = slice(c * E_PER, (c + 1) * E_PER)
        in_maps.append({"xs": xs[sl], "w1s": w1s[sl], "w2s": w2s[sl]})
    return in_maps


def run(inputs, trace=False):
    """Returns (y_full, BassKernelResults)."""
    in_maps = _prep_inputs(inputs["x"], inputs["batched_fc1_w"],
                           inputs["batched_fc2_w"])
    nc = get_nc()
    res = run_bass_kernel_spmd(nc, in_maps, list(range(N_CORES)), trace=trace)
    y_full = np.concatenate([res.results[c]["y"] for c in range(N_CORES)],
                            axis=0)
    return y_full, res


def kernel(x, fwd_expert_count, batched_fc1_w, batched_fc2_w):
    y, _ = run({"x": x, "batched_fc1_w": batched_fc1_w,
                "batched_fc2_w": batched_fc2_w})
    return y


# revision 2
# speedup vs baseline: 1.0480x; 1.0480x over previous
# MoE grouped-GEMM kernel for Trainium2 (Bass/Tile), 8 NeuronCores SPMD.
#
# Problem: x [65536, 1024] fp32, 64 experts each owning a contiguous group of
# 1024 tokens. Per expert: h = relu(x_g @ W1^T) (1024->64), y_g = h @ W2^T
# (64->1024).
#
# Sharding: expert-parallel == token-parallel here (tokens pre-sorted by
# expert, equal groups). Core c handles experts 8c..8c+7 and their 8192
# tokens. No collectives needed; host slices inputs and concatenates outputs.
#
# Per-core kernel (memory-bound; ~68 MB HBM traffic/core):
#   loads stream on the nc.sync HWDGE ring; stores go out on the
#   nc.gpsimd SWDGE path so a store waiting on compute never blocks the
#   next expert's x load.
import numpy as np

import concourse.bacc as bacc
import concourse.bass as bass
import concourse.mybir as mybir
import concourse.tile as tile
from concourse.bass import ds, ts
from concourse.bass_utils import run_bass_kernel_spmd

E = 64          # experts
H = 64          # expert hidden
D = 1024        # d_in
O = 1024        # d_out
T = 65536       # total tokens
N_CORES = 8
E_PER = E // N_CORES        # 8 experts per core
TPE = T // E                # 1024 tokens per expert
T_PER = TPE * E_PER         # 8192 tokens per core
DC = D // 128               # 8 contraction chunks of 128
MC = TPE // 128             # 8 token chunks of 128 per expert
FP32 = mybir.dt.float32

_NC_CACHE = {}


def build_nc():
    nc = bacc.Bacc("TRN2", target_bir_lowering=False, debug=False,
                   num_devices=N_CORES)

    xs = nc.declare_dram_parameter("xs", [E_PER, 128, DC, TPE], FP32,
                                   isOutput=False)
    w1s = nc.declare_dram_parameter("w1s", [E_PER, 128, DC * H], FP32,
                                    isOutput=False)
    w2s = nc.declare_dram_parameter("w2s", [E_PER, H, O], FP32,
                                    isOutput=False)
    y = nc.declare_dram_parameter("y", [T_PER, O], FP32, isOutput=True)

    with tile.TileContext(nc) as tc:
        with (
            tc.tile_pool(name="w1pool", bufs=1) as w1pool,
            tc.tile_pool(name="w2pool", bufs=2) as w2pool,
            tc.tile_pool(name="xpool", bufs=6) as xpool,
            tc.tile_pool(name="hpool", bufs=2) as hpool,
            tc.tile_pool(name="ypool", bufs=4) as ypool,
            tc.tile_pool(name="phpool", bufs=2, space=bass.MemorySpace.PSUM) as phpool,
            tc.tile_pool(name="pypool", bufs=2, space=bass.MemorySpace.PSUM) as pypool,
        ):
            w1_t = w1pool.tile([128, E_PER, DC * H], FP32, tag="w1")
            w2_ts = []
            x_ts = []

            def load_expert(le):
                # x in two 2 MB halves (dc 0..3 / 4..7) so FC1 can start
                # on the first half and slots recycle finer.
                xa = xpool.tile([128, DC // 2, TPE], FP32, tag="x")
                xb = xpool.tile([128, DC // 2, TPE], FP32, tag="x")
                nc.sync.dma_start(out=xa[:], in_=xs[le, :, 0:DC // 2, :])
                nc.sync.dma_start(out=xb[:], in_=xs[le, :, DC // 2:DC, :])
                w2_t = w2pool.tile([H, O], FP32, tag="w2")
                nc.sync.dma_start(out=w2_t[:], in_=w2s[le])
                x_ts.append((xa, xb))
                w2_ts.append(w2_t)

            # Prime the pipeline: x(0) first, then all FC1 weights (one
            # 2 MB DMA), then x(1), x(2).
            load_expert(0)
            nc.sync.dma_start(out=w1_t[:],
                              in_=w1s[:].rearrange("e p f -> p e f"))
            load_expert(1)
            load_expert(2)

            for le in range(E_PER):
                if le + 3 < E_PER:
                    load_expert(le + 3)
                xa, xb = x_ts[le]
                w2_t = w2_ts[le]

                # FC1: psum_h[h, t] = sum_d W1[h, d] * x[t, d]
                ph = phpool.tile([H, TPE], FP32, tag="ph")
                for tb in range(TPE // 512):
                    for dc in range(DC):
                        xt = xa if dc < DC // 2 else xb
                        nc.tensor.matmul(
                            ph[:, ts(tb, 512)],
                            w1_t[:, le, ds(dc * H, H)],      # lhsT [128, 64]
                            xt[:, dc % (DC // 2), ts(tb, 512)],  # rhs [128, 512]
                            start=(dc == 0),
                            stop=(dc == DC - 1),
                        )

                h_t = hpool.tile([H, TPE], FP32, tag="h")
                nc.scalar.activation(h_t[:], ph[:],
                                     mybir.ActivationFunctionType.Relu)

                # FC2: y[t, o] = sum_h h[h, t] * W2[o, h]; store in two
                # 2 MB chunks on the SWDGE (gpsimd) path.
                for half in range(2):
                    y_t = ypool.tile([128, MC // 2, O], FP32, tag="y")
                    for mm in range(MC // 2):
                        m = half * (MC // 2) + mm
                        py = pypool.tile([128, O], FP32, tag="py")
                        for oc in range(O // 512):
                            nc.tensor.matmul(
                                py[:, ts(oc, 512)],
                                h_t[:, ts(m, 128)],          # lhsT [64, 128]
                                w2_t[:, ts(oc, 512)],        # rhs  [64, 512]
                                start=True,
                                stop=True,
                            )
                        nc.vector.tensor_copy(y_t[:, mm, :], py[:])

                    y_view = y[ds(le * TPE + half * (TPE // 2), TPE // 2),
                               :].rearrange("(m p) o -> p m o", p=128)
                    nc.gpsimd.dma_start(out=y_view, in_=y_t[:])

    nc.compile()
    return nc


def get_nc():
    if "nc" not in _NC_CACHE:
        _NC_CACHE["nc"] = build_nc()
    return _NC_CACHE["nc"]


def _prep_inputs(x, batched_fc1_w, batched_fc2_w):
    x = np.ascontiguousarray(np.asarray(x, dtype=np.float32))
    fc1 = np.ascontiguousarray(np.asarray(batched_fc1_w, dtype=np.float32))
    fc2 = np.ascontiguousarray(np.asarray(batched_fc2_w, dtype=np.float32))

    # xs[e, p, c, t] = x[e*TPE + t, c*128 + p]
    xs = np.ascontiguousarray(
        x.reshape(E, TPE, DC, 128).transpose(0, 3, 2, 1))
    # w1s[e, p, c*H + h] = W1[e, h, c*128 + p]
    w1s = np.ascontiguousarray(
        fc1.reshape(E, H, DC, 128).transpose(0, 3, 2, 1).reshape(E, 128, DC * H))
    # w2s[e, h, o] = W2[e, o, h]
    w2s = np.ascontiguousarray(fc2.transpose(0, 2, 1))

    in_maps = []
    for c in range(N_CORES):
        sl = slice(c * E_PER, (c + 1) * E_PER)
        in_maps.append({"xs": xs[sl], "w1s": w1s[sl], "w2s": w2s[sl]})
    return in_maps


def run(inputs, trace=False):
    """Returns (y_full, BassKernelResults)."""
    in_maps = _prep_inputs(inputs["x"], inputs["batched_fc1_w"],
                           inputs["batched_fc2_w"])
    nc = get_nc()
    res = run_bass_kernel_spmd(nc, in_maps, list(range(N_CORES)), trace=trace)
    y_full = np.concatenate([res.results[c]["y"] for c in range(N_CORES)],
                            axis=0)
    return y_full, res


def kernel(x, fwd_expert_count, batched_fc1_w, batched_fc2_w):
    y, _ = run({"x": x, "batched_fc1_w": batched_fc1_w,
                "batched_fc2_w": batched_fc2_w})
    return y


# revision 3
# speedup vs baseline: 1.3078x; 1.2479x over previous
# MoE grouped-GEMM kernel for Trainium2 (Bass/Tile), 8 NeuronCores SPMD.
#
# Problem: x [65536, 1024] fp32, 64 experts each owning a contiguous group of
# 1024 tokens. Per expert: h = relu(x_g @ W1^T) (1024->64), y_g = h @ W2^T
# (64->1024).
#
# Sharding: expert-parallel == token-parallel here (tokens pre-sorted by
# expert, equal groups). Core c handles experts 8c..8c+7 and their 8192
# tokens. No collectives needed; host slices inputs and concatenates outputs.
#
# Per-core kernel (memory-bound; ~68 MB HBM traffic/core):
#   loads stream on the nc.sync HWDGE ring; stores go out on the
#   nc.gpsimd SWDGE path so a store waiting on compute never blocks the
#   next expert's x load.
import numpy as np

import concourse.bacc as bacc
import concourse.bass as bass
import concourse.mybir as mybir
import concourse.tile as tile
from concourse.bass import ds, ts
from concourse.bass_utils import run_bass_kernel_spmd

E = 64          # experts
H = 64          # expert hidden
D = 1024        # d_in
O = 1024        # d_out
T = 65536       # total tokens
N_CORES = 8
E_PER = E // N_CORES        # 8 experts per core
TPE = T // E                # 1024 tokens per expert
T_PER = TPE * E_PER         # 8192 tokens per core
DC = D // 128               # 8 contraction chunks of 128
MC = TPE // 128             # 8 token chunks of 128 per expert
FP32 = mybir.dt.float32
FP32R = mybir.dt.float32r

_NC_CACHE = {}


def build_nc():
    nc = bacc.Bacc("TRN2", target_bir_lowering=False, debug=False,
                   num_devices=N_CORES)

    xs = nc.declare_dram_parameter("xs", [E_PER, 128, DC, TPE], FP32R,
                                   isOutput=False)
    w1s = nc.declare_dram_parameter("w1s", [E_PER, 128, DC * H], FP32R,
                                    isOutput=False)
    w2s = nc.declare_dram_parameter("w2s", [E_PER, H, O], FP32R,
                                    isOutput=False)
    y = nc.declare_dram_parameter("y", [T_PER, O], FP32, isOutput=True)

    with tile.TileContext(nc) as tc:
        with (
            tc.tile_pool(name="w1pool", bufs=1) as w1pool,
            tc.tile_pool(name="w2pool", bufs=2) as w2pool,
            tc.tile_pool(name="xpool", bufs=6) as xpool,
            tc.tile_pool(name="hpool", bufs=2) as hpool,
            tc.tile_pool(name="ypool", bufs=4) as ypool,
            tc.tile_pool(name="phpool", bufs=2, space=bass.MemorySpace.PSUM) as phpool,
            tc.tile_pool(name="pypool", bufs=2, space=bass.MemorySpace.PSUM) as pypool,
        ):
            w1_t = w1pool.tile([128, E_PER, DC * H], FP32R, tag="w1")
            w2_ts = []
            x_ts = []

            def load_expert(le):
                # x in two 2 MB halves (dc 0..3 / 4..7) so FC1 can start
                # on the first half and slots recycle finer.
                xa = xpool.tile([128, DC // 2, TPE], FP32R, tag="x")
                xb = xpool.tile([128, DC // 2, TPE], FP32R, tag="x")
                nc.sync.dma_start(out=xa[:], in_=xs[le, :, 0:DC // 2, :])
                nc.sync.dma_start(out=xb[:], in_=xs[le, :, DC // 2:DC, :])
                w2_t = w2pool.tile([H, O], FP32R, tag="w2")
                nc.sync.dma_start(out=w2_t[:], in_=w2s[le])
                x_ts.append((xa, xb))
                w2_ts.append(w2_t)

            # Prime the pipeline: x(0) first, then all FC1 weights (one
            # 2 MB DMA), then x(1), x(2).
            load_expert(0)
            nc.sync.dma_start(out=w1_t[:],
                              in_=w1s[:].rearrange("e p f -> p e f"))
            load_expert(1)
            load_expert(2)

            for le in range(E_PER):
                if le + 3 < E_PER:
                    load_expert(le + 3)
                xa, xb = x_ts[le]
                w2_t = w2_ts[le]

                # FC1: psum_h[h, t] = sum_d W1[h, d] * x[t, d]
                ph = phpool.tile([H, TPE], FP32, tag="ph")
                for tb in range(TPE // 512):
                    for dc in range(DC):
                        xt = xa if dc < DC // 2 else xb
                        nc.tensor.matmul(
                            ph[:, ts(tb, 512)],
                            w1_t[:, le, ds(dc * H, H)],      # lhsT [128, 64]
                            xt[:, dc % (DC // 2), ts(tb, 512)],  # rhs [128, 512]
                            start=(dc == 0),
                            stop=(dc == DC - 1),
                        )

                h_t = hpool.tile([H, TPE], FP32R, tag="h")
                nc.scalar.activation(h_t[:], ph[:],
                                     mybir.ActivationFunctionType.Relu)

                # FC2: y[t, o] = sum_h h[h, t] * W2[o, h]; store in two
                # 2 MB chunks on the SWDGE (gpsimd) path.
                for half in range(2):
                    y_t = ypool.tile([128, MC // 2, O], FP32, tag="y")
                    for mm in range(MC // 2):
                        m = half * (MC // 2) + mm
                        py = pypool.tile([128, O], FP32, tag="py")
                        for oc in range(O // 512):
                            nc.tensor.matmul(
                                py[:, ts(oc, 512)],
                                h_t[:, ts(m, 128)],          # lhsT [64, 128]
                                w2_t[:, ts(oc, 512)],        # rhs  [64, 512]
                                start=True,
                                stop=True,
                            )
                        nc.vector.tensor_copy(y_t[:, mm, :], py[:])

                    y_view = y[ds(le * TPE + half * (TPE // 2), TPE // 2),
                               :].rearrange("(m p) o -> p m o", p=128)
                    nc.gpsimd.dma_start(out=y_view, in_=y_t[:])

    nc.compile()
    return nc


def get_nc():
    if "nc" not in _NC_CACHE:
        _NC_CACHE["nc"] = build_nc()
    return _NC_CACHE["nc"]


def _prep_inputs(x, batched_fc1_w, batched_fc2_w):
    x = np.ascontiguousarray(np.asarray(x, dtype=np.float32))
    fc1 = np.ascontiguousarray(np.asarray(batched_fc1_w, dtype=np.float32))
    fc2 = np.ascontiguousarray(np.asarray(batched_fc2_w, dtype=np.float32))

    # xs[e, p, c, t] = x[e*TPE + t, c*128 + p]
    xs = np.ascontiguousarray(
        x.reshape(E, TPE, DC, 128).transpose(0, 3, 2, 1))
    # w1s[e, p, c*H + h] = W1[e, h, c*128 + p]
    w1s = np.ascontiguousarray(
        fc1.reshape(E, H, DC, 128).transpose(0, 3, 2, 1).reshape(E, 128, DC * H))
    # w2s[e, h, o] = W2[e, o, h]
    w2s = np.ascontiguousarray(fc2.transpose(0, 2, 1))

    in_maps = []
    for c in range(N_CORES):
        sl = slice(c * E_PER, (c + 1) * E_PER)
        in_maps.append({"xs": xs[sl], "w1s": w1s[sl], "w2s": w2s[sl]})
    return in_maps


def run(inputs, trace=False):
    """Returns (y_full, BassKernelResults)."""
    in_maps = _prep_inputs(inputs["x"], inputs["batched_fc1_w"],
                           inputs["batched_fc2_w"])
    nc = get_nc()
    res = run_bass_kernel_spmd(nc, in_maps, list(range(N_CORES)), trace=trace)
    y_full = np.concatenate([res.results[c]["y"] for c in range(N_CORES)],
                            axis=0)
    return y_full, res


def kernel(x, fwd_expert_count, batched_fc1_w, batched_fc2_w):
    y, _ = run({"x": x, "batched_fc1_w": batched_fc1_w,
                "batched_fc2_w": batched_fc2_w})
    return y
